# revision 16
# baseline (speedup 1.0000x reference)
"""Grouped-channel attention (CAT FullAttention) Trainium2 kernel.

Math (per batch element b; L=S=96, R=70, E=10, P=7):
  scores[l,s,p,r] = sum_e q[l,e,p] * k[s,e,r]
  A = softmax over (s,p) of scores           (per l, r)
  out[l,e,r]      = sum_{s,p} v[s,e,p] * A[l,s,p,r]

Strategy: pure data parallel over the batch dim (B=256 -> 32 per core x 8
cores). Per batch element on-device:
  e1   (PE) : per r, scores[s,(p,l)] = K_r^T @ Q2, fp32r matmuls, N=336
              chunks written to 512-element-aligned PSUM slots.
  exp       : split across two engines so neither is the bottleneck —
              ACT exp for slots 0-7 (the r0-3 group), DVE for slots 8-13
              (r4-6) via the Schraudolph bit trick: fp32 A bits =
              int32(x * 2^23/ln2 + (127*2^23 - C)).  C centers the
              multiplicative sawtooth at 1 (+-3%); the constant factor
              cancels exactly between softmax numerator and denominator.
  e2   (PE) : per p, E[e',(r,l)] += V_p^T @ A_p accumulated over p in PSUM.
              V carries a ones-channel at e'=10, so E[10,:] is the softmax
              denominator.
  tail      : PSUM evacuation on GPSIMD (Pool), transpose E to [l,(r,e')]
              on PE, reciprocal+broadcast multiply on DVE, contiguous DMA
              of [96,70] per batch.
"""

import sys

if "/opt/trn_rl_repo" not in sys.path:
    sys.path.insert(0, "/opt/trn_rl_repo")

import ml_dtypes
import numpy as np

import concourse.bass as bass
import concourse.bacc as bacc
import concourse.tile as tile
from concourse import mybir, masks
from concourse.bass_utils import run_bass_kernel_spmd

B, L, R = 256, 96, 70
E, P = 10, 7
EP = E + 1  # v channels + ones channel
NCORES = 8
BPC = B // NCORES  # batches per core
G = 4  # batches per DMA group
F32R = mybir.dt.float32r
F32 = mybir.dt.float32
BF16 = mybir.dt.bfloat16
I16 = mybir.dt.int16

# Schraudolph exp in bf16 bits: bits = int16(x * SCH_A + SCH_B) read as
# bf16 gives exp(x) * g, g in [1/1.0303, 1.0303] (centered sawtooth,
# period ln2). The constant factor cancels between softmax num and den.
SCH_A = 184.66428386431385  # 2^7 / ln 2
SCH_B = 16256.0 - 5.5112  # 127 * 2^7 - 2^7*log2(sqrt(1.061451))

_CACHE = {}


def _build(bpc, repeat=1):
    nc = bacc.Bacc("TRN2", target_bir_lowering=False, debug=False, num_devices=NCORES)
    q_d = nc.dram_tensor("q2", [bpc, E, P * L], F32R, kind="ExternalInput").ap()
    k_d = nc.dram_tensor("kt", [bpc, E, P * L], F32R, kind="ExternalInput").ap()
    v_d = nc.dram_tensor("vt", [bpc, L, P * EP], BF16, kind="ExternalInput").ap()
    o_d = nc.dram_tensor("out", [bpc, L, R], F32, kind="ExternalOutput").ap()

    ngroups = bpc // G
    CH = 336  # e1 chunk width: (p,l)=672 split in two, each >=256 for fp32r
    SLOT = 512  # psum chunk slot (one bank)
    # chunk-tile packing: one r per 2-slot tile; slots 0-7 exp'd on ACT
    # (feeds e2 group r0-3), slots 8-13 on DVE via Schraudolph (r4-6).
    # 2-slot tiles with bufs=3 keep 6 PSUM banks but give the PE 3-deep
    # slack over the exp engines, hiding the exp+semaphore latency.
    TILES = [(0, 2, "a"), (2, 2, "a"), (4, 2, "a"), (6, 2, "a"),
             (8, 2, "d"), (10, 2, "d"), (12, 2, "d")]
    # batch 0 leads with a 1-slot region so the first exp fires one cold
    # matmul after the DMA instead of two
    TILES0 = [(0, 1, "a"), (1, 2, "a"), (3, 2, "a"), (5, 2, "a"), (7, 1, "a"),
              (8, 2, "d"), (10, 2, "d"), (12, 2, "d")]

    with tile.TileContext(nc) as tc:
        with (
            tc.tile_pool(name="const", bufs=1) as cpool,
            tc.tile_pool(name="qk", bufs=2) as qkpool,
            tc.tile_pool(name="apool", bufs=3) as apool,
            tc.tile_pool(name="esb", bufs=2) as epool,
            tc.tile_pool(name="rd", bufs=2) as rdpool,
            tc.tile_pool(name="og", bufs=2) as ogpool,
            tc.tile_pool(name="ps_s", bufs=3, space="PSUM") as spool,
            tc.tile_pool(name="ps_e", bufs=1, space="PSUM") as e2pool,
        ):
            ident = cpool.tile([EP, EP], F32)
            masks.make_identity(nc, ident[:])
            # touch Exp once so the ACT table set loads before the exps;
            # it shares ACT with the two leading DMAs issued below
            warm = cpool.tile([1, 1], F32)
            nc.scalar.activation(
                warm[:], ident[0:1, 0:1], mybir.ActivationFunctionType.Exp
            )

            # per-group input tiles, per-batch A tiles
            vg = [None] * ngroups
            A = [None] * bpc
            Et = [None] * bpc
            Eb = [None] * bpc
            OG = [None] * ngroups

            QS = [None] * bpc
            KS = [None] * bpc

            def load_group(g):
                    g0 = g * G
                    if g == 0:
                        # batch 0 gets its own tiles so the first matmuls
                        # depend only on the two small leading DMAs
                        qb0 = qkpool.tile([E, P * L], F32R, tag="qb0")
                        kb0 = qkpool.tile([E, P * L], F32R, tag="kb0")
                        # ACT's own HWDGE queue: dispatches in parallel
                        # with SP's and frees the first matmuls from the
                        # SP queue's in-order completion semaphore
                        nc.scalar.dma_start(kb0[:], k_d[0])
                        nc.scalar.dma_start(qb0[:], q_d[0])
                        qt = qkpool.tile([E, (G - 1) * P * L], F32R, tag="qg")
                        kt = qkpool.tile([E, (G - 1) * P * L], F32R, tag="kg")
                        nc.sync.dma_start(
                            qt[:].rearrange("e (b f) -> e b f", b=G - 1),
                            q_d[1:G].rearrange("b e f -> e b f"),
                        )
                        nc.sync.dma_start(
                            kt[:].rearrange("e (b f) -> e b f", b=G - 1),
                            k_d[1:G].rearrange("b e f -> e b f"),
                        )
                        QS[0], KS[0] = qb0[:], kb0[:]
                        for i in range(1, G):
                            QS[i] = qt[:, (i - 1) * 672 : i * 672]
                            KS[i] = kt[:, (i - 1) * 672 : i * 672]
                    else:
                        qt = qkpool.tile([E, G * P * L], F32R, tag="qg")
                        kt = qkpool.tile([E, G * P * L], F32R, tag="kg")
                        nc.sync.dma_start(
                            qt[:].rearrange("e (b f) -> e b f", b=G),
                            q_d[g0 : g0 + G].rearrange("b e f -> e b f"),
                        )
                        nc.sync.dma_start(
                            kt[:].rearrange("e (b f) -> e b f", b=G),
                            k_d[g0 : g0 + G].rearrange("b e f -> e b f"),
                        )
                        for i in range(G):
                            QS[g0 + i] = qt[:, i * 672 : (i + 1) * 672]
                            KS[g0 + i] = kt[:, i * 672 : (i + 1) * 672]
                    vt = qkpool.tile([L, G * P * EP], BF16, tag="vg")
                    nc.sync.dma_start(
                        vt[:].rearrange("s (b f) -> s b f", b=G),
                        v_d[g0 : g0 + G].rearrange("b s f -> s b f"),
                    )
                    vg[g] = vt

            def stage1_tiles(b):
                """e1 matmuls + exp (ACT or DVE per tile) into A[b].

                Generator: yields after each tile so e2 work of the
                previous batch can interleave — the PE would otherwise
                outrun the exp engines' 745-825ns/tile drain rate and
                stall on the 3-deep PSUM window."""
                qs, ks = QS[b], KS[b]
                at = apool.tile([L, P * P * L], BF16, tag="A")
                A[b] = at
                for j0, nch, eng in TILES0 if b == 0 else TILES:
                    st = spool.tile([L, 2 * SLOT], F32, tag="s")
                    for m in range(nch):
                        jj = j0 + m
                        r, c = divmod(jj, 2)
                        nc.tensor.matmul(
                            st[:, m * SLOT : m * SLOT + CH],
                            lhsT=ks[:, r * L : (r + 1) * L],
                            rhs=qs[:, c * CH : (c + 1) * CH],
                            start=True,
                            stop=True,
                        )
                    src = st[:, 0 : nch * SLOT].rearrange(
                        "s (a c) -> s a c", c=SLOT
                    )[:, :, 0:CH]
                    dst = at[:, j0 * CH : (j0 + nch) * CH].rearrange(
                        "s (a c) -> s a c", c=CH
                    )
                    if eng == "a":
                        nc.scalar.activation(
                            dst, src, mybir.ActivationFunctionType.Exp
                        )
                    else:
                        nc.vector.tensor_scalar(
                            dst.bitcast(I16), src, SCH_A, SCH_B,
                            mybir.AluOpType.mult, mybir.AluOpType.add,
                        )
                    yield

            def stage2_chunks(b):
                """e2 accumulation over p in PSUM (generator, 4 chunks)."""
                g, i = divmod(b, G)
                vt = vg[g]
                at = A[b]
                a3 = at[:].rearrange("s (r f) -> s r f", f=P * L)
                # two separate 1-bank accumulators: the r0-3 group's exps
                # (ACT) and the r4-6 group's (DVE) finish independently
                et0 = e2pool.tile([EP, 384], F32, tag="e2a")
                et1 = e2pool.tile([EP, 288], F32, tag="e2b")
                Et[b] = (et0, et1)
                o0 = et0[:].rearrange("e (r l) -> e r l", l=L)
                o1 = et1[:].rearrange("e (r l) -> e r l", l=L)

                def e2_part(out, r0, r1, p0, p1):
                    for p in range(p0, p1):
                        nc.tensor.matmul(
                            out,
                            lhsT=vt[:, i * 77 + p * EP : i * 77 + (p + 1) * EP],
                            rhs=a3[:, r0:r1, p * L : (p + 1) * L],
                            start=(p == 0),
                            stop=(p == P - 1),
                        )

                # o1 first for the last batch so its evacuation+transposes
                # overlap the o0 matmuls in the drain
                groups = [(o1, 4, 7), (o0, 0, 4)] if b == bpc - 1 else [
                    (o0, 0, 4), (o1, 4, 7)]
                for out, r0, r1 in groups:
                    e2_part(out, r0, r1, 0, 4)
                    yield
                    e2_part(out, r0, r1, 4, P)
                    yield

            def stage3(b):
                g, i = divmod(b, G)
                et0, et1 = Et[b]

                # separate tiles so the r0-3 transposes depend only on the
                # et0 evacuation (and vice versa); GPSIMD can't read PSUM,
                # so the copies split across the two exp engines (Copy
                # shares the ACT table set with Exp)
                eb0 = epool.tile([EP, 4 * L], F32, tag="eb0")
                eb1 = epool.tile([EP, 3 * L], F32, tag="eb1")
                last = b == bpc - 1
                if not last:
                    nc.scalar.activation(
                        eb0[:], et0[:], mybir.ActivationFunctionType.Copy
                    )
                    nc.vector.tensor_copy(eb1[:], et1[:])
                else:
                    # last batch ran e2 o1-first; evacuate in that order
                    nc.vector.tensor_copy(eb1[:], et1[:])
                    nc.scalar.activation(
                        eb0[:], et0[:], mybir.ActivationFunctionType.Copy
                    )
                if b >= bpc - 2:
                    # scores pool is idle for the final batches; keep the
                    # e2 accumulator slot free so the last e2 starts sooner
                    tt = spool.tile([L, P * EP], F32, tag="s")
                else:
                    tt = e2pool.tile([L, P * EP], F32, tag="e2a")

                def transpose_half(rs):
                    for r in rs:
                        src_ = (
                            eb0[:, r * L : (r + 1) * L]
                            if r < 4
                            else eb1[:, (r - 4) * L : (r - 3) * L]
                        )
                        nc.tensor.transpose(
                            tt[:, r * EP : (r + 1) * EP], src_, ident[:]
                        )

                if last:
                    transpose_half(range(4, P))
                    transpose_half(range(4))
                else:
                    transpose_half(range(P))
                t3 = tt[:].rearrange("l (r e) -> l r e", e=EP)
                rd = rdpool.tile([L, P], F32, tag="rd")
                r3 = rd[:].rearrange("l (r u) -> l r u", u=1)
                nc.vector.reciprocal(r3, t3[:, :, E : E + 1])
                if OG[g] is None:
                    og_tile = ogpool.tile([L, G * R], F32, tag="og")
                    OG[g] = og_tile
                og = OG[g]
                dst = og[:, i * R : (i + 1) * R].rearrange("l (e r) -> l r e", r=P)
                rdb = r3.copy()
                rdb.ap = rdb.ap[:-1] + [[0, E]]
                nc.vector.tensor_mul(dst, t3[:, :, 0:E], rdb)

            def flush_group(g):
                g0 = g * G
                if g == ngroups - 1:
                    # split the final flush so the tail only waits on the
                    # last batch's slice
                    nc.sync.dma_start(
                        o_d[g0 : g0 + G - 1].rearrange("b l c -> l b c"),
                        OG[g][:, 0 : (G - 1) * R].rearrange(
                            "l (b c) -> l b c", b=G - 1
                        ),
                    )
                    nc.sync.dma_start(
                        o_d[g0 + G - 1], OG[g][:, (G - 1) * R : G * R]
                    )
                else:
                    nc.sync.dma_start(
                        o_d[g0 : g0 + G].rearrange("b l c -> l b c"),
                        OG[g][:].rearrange("l (b c) -> l b c", b=G),
                    )
                OG[g] = None

            # software pipeline: stage1(b) tiles interleaved with e2
            # chunks of batch b-1 so the exp engines never starve the PE
            def drain(it):
                if it is not None:
                    for _ in it:
                        pass

            for rep in range(repeat):
                load_group(0)
                for b in range(bpc + 1):
                    s1 = s2 = None
                    if b < bpc:
                        g, i = divmod(b, G)
                        if i == 0 and g + 1 < ngroups:
                            load_group(g + 1)
                        s1 = stage1_tiles(b)
                    if b >= 1:
                        s2 = stage2_chunks(b - 1)
                    if s1 is not None:
                        # lead with 3 e1 tiles, then alternate chunk/tile
                        for _ in range(3):
                            next(s1, None)
                        if s2 is not None:
                            for _ in s2:
                                next(s1, None)
                        for _ in s1:
                            pass
                    else:
                        drain(s2)
                    if b >= 1:
                        stage3(b - 1)
                        if (b - 1) % G == G - 1:
                            flush_group((b - 1) // G)

    nc.compile()
    return nc


def _get_nc(bpc=BPC, repeat=1):
    key = (bpc, repeat)
    if key not in _CACHE:
        _CACHE[key] = _build(bpc, repeat)
    return _CACHE[key]


def _prep(queries, keys, values):
    q = np.asarray(queries, dtype=np.float32)
    k = np.asarray(keys, dtype=np.float32)
    v = np.asarray(values, dtype=np.float32)
    b = q.shape[0]
    # Q2[b, e, p*96+l] = q[b, l, e*7+p]
    q2 = np.ascontiguousarray(
        q.reshape(b, L, E, P).transpose(0, 2, 3, 1).reshape(b, E, P * L)
    )
    # KT[b, e, r*96+s] = k[b, s, e*7+r]
    kt = np.ascontiguousarray(
        k.reshape(b, L, E, P).transpose(0, 2, 3, 1).reshape(b, E, P * L)
    )
    # VT[b, s, p*11+e'] = v[b, s, e'*7+p] for e'<10, 1.0 at e'=10
    v4 = v.reshape(b, L, E, P).transpose(0, 1, 3, 2)  # [b, s, p, e]
    vt = np.concatenate([v4, np.ones((b, L, P, 1), np.float32)], axis=-1)
    vt = np.ascontiguousarray(
        vt.reshape(b, L, P * EP).astype(ml_dtypes.bfloat16)
    )
    return q2, kt, vt


def kernel(queries, keys, values, attn_mask=None, _trace=False):
    nc = _get_nc()
    q2, kt, vt = _prep(queries, keys, values)
    in_maps = []
    for c in range(NCORES):
        s = slice(c * BPC, (c + 1) * BPC)
        in_maps.append({"q2": q2[s], "kt": kt[s], "vt": vt[s]})
    res = None
    for attempt in range(3):
        try:
            res = run_bass_kernel_spmd(
                nc, in_maps, core_ids=list(range(NCORES)), trace=_trace
            )
            break
        except Exception:
            # shared terminal occasionally reports transient NRT device
            # errors; back off and retry
            if attempt == 2:
                raise
            import time as _time

            _time.sleep(15)
    out = np.concatenate([res.results[c]["out"] for c in range(NCORES)], axis=0)
    if _trace:
        kernel.last_exec_time_ns = res.exec_time_ns
        kernel.last_results = res
    return out.astype(np.float32)


# revision 17
# speedup vs baseline: 1.0452x; 1.0452x over previous
"""Grouped-channel attention (CAT FullAttention) Trainium2 kernel.

Math (per batch element b; L=S=96, R=70, E=10, P=7):
  scores[l,s,p,r] = sum_e q[l,e,p] * k[s,e,r]
  A = softmax over (s,p) of scores           (per l, r)
  out[l,e,r]      = sum_{s,p} v[s,e,p] * A[l,s,p,r]

Strategy: pure data parallel over the batch dim (B=256 -> 32 per core x 8
cores). Per batch element on-device:
  e1   (PE) : per r, scores[s,(p,l)] = K_r^T @ Q2, fp32r matmuls, N=336
              chunks written to 512-element-aligned PSUM slots.
  exp       : split across two engines so neither is the bottleneck —
              ACT exp for slots 0-7 (the r0-3 group), DVE for slots 8-13
              (r4-6) via the Schraudolph bit trick: fp32 A bits =
              int32(x * 2^23/ln2 + (127*2^23 - C)).  C centers the
              multiplicative sawtooth at 1 (+-3%); the constant factor
              cancels exactly between softmax numerator and denominator.
  e2   (PE) : per p, E[e',(r,l)] += V_p^T @ A_p accumulated over p in PSUM.
              V carries a ones-channel at e'=10, so E[10,:] is the softmax
              denominator.
  tail      : PSUM evacuation on GPSIMD (Pool), transpose E to [l,(r,e')]
              on PE, reciprocal+broadcast multiply on DVE, contiguous DMA
              of [96,70] per batch.
"""

import sys

if "/opt/trn_rl_repo" not in sys.path:
    sys.path.insert(0, "/opt/trn_rl_repo")

import ml_dtypes
import numpy as np

import concourse.bass as bass
import concourse.bacc as bacc
import concourse.tile as tile
from concourse import mybir, masks
from concourse.bass_utils import run_bass_kernel_spmd

B, L, R = 256, 96, 70
E, P = 10, 7
EP = E + 1  # v channels + ones channel
NCORES = 8
BPC = B // NCORES  # batches per core
G = 4  # batches per DMA group
F32R = mybir.dt.float32r
F32 = mybir.dt.float32
BF16 = mybir.dt.bfloat16
I16 = mybir.dt.int16

# Schraudolph exp in bf16 bits: bits = int16(x * SCH_A + SCH_B) read as
# bf16 gives exp(x) * g, g in [1/1.0303, 1.0303] (centered sawtooth,
# period ln2). The constant factor cancels between softmax num and den.
SCH_A = 184.66428386431385  # 2^7 / ln 2
SCH_B = 16256.0 - 5.5112  # 127 * 2^7 - 2^7*log2(sqrt(1.061451))

_CACHE = {}


def _build(bpc, repeat=1):
    nc = bacc.Bacc("TRN2", target_bir_lowering=False, debug=False, num_devices=NCORES)
    q_d = nc.dram_tensor("q2", [bpc, E, P * L], F32R, kind="ExternalInput").ap()
    k_d = nc.dram_tensor("kt", [bpc, E, P * L], F32R, kind="ExternalInput").ap()
    v_d = nc.dram_tensor("vt", [bpc, L, P * EP], BF16, kind="ExternalInput").ap()
    o_d = nc.dram_tensor("out", [bpc, L, R], F32, kind="ExternalOutput").ap()

    ngroups = bpc // G
    CH = 336  # e1 chunk width: (p,l)=672 split in two, each >=256 for fp32r
    SLOT = 512  # psum chunk slot (one bank)
    # chunk-tile packing: one r per 2-slot tile; slots 0-7 exp'd on ACT
    # (feeds e2 group r0-3), slots 8-13 on DVE via Schraudolph (r4-6).
    # 2-slot tiles with bufs=3 keep 6 PSUM banks but give the PE 3-deep
    # slack over the exp engines, hiding the exp+semaphore latency.
    TILES = [(0, 2, "a"), (2, 2, "a"), (4, 2, "a"), (6, 2, "a"),
             (8, 2, "d"), (10, 2, "d"), (12, 2, "d")]
    # batch 0 leads with a 1-slot region so the first exp fires one cold
    # matmul after the DMA instead of two
    TILES0 = [(0, 1, "a"), (1, 2, "a"), (3, 2, "a"), (5, 2, "a"), (7, 1, "a"),
              (8, 2, "d"), (10, 2, "d"), (12, 2, "d")]

    with tile.TileContext(nc) as tc:
        with (
            tc.tile_pool(name="const", bufs=1) as cpool,
            tc.tile_pool(name="qk", bufs=2) as qkpool,
            tc.tile_pool(name="apool", bufs=3) as apool,
            tc.tile_pool(name="esb", bufs=2) as epool,
            tc.tile_pool(name="rd", bufs=2) as rdpool,
            tc.tile_pool(name="og", bufs=2) as ogpool,
            tc.tile_pool(name="ps_s", bufs=3, space="PSUM") as spool,
            tc.tile_pool(name="ps_e", bufs=1, space="PSUM") as e2pool,
        ):
            ident = cpool.tile([EP, EP], F32)
            masks.make_identity(nc, ident[:])
            # touch Exp once so the ACT table set loads before the exps;
            # it shares ACT with the two leading DMAs issued below
            warm = cpool.tile([1, 1], F32)
            nc.scalar.activation(
                warm[:], ident[0:1, 0:1], mybir.ActivationFunctionType.Exp
            )

            # per-group input tiles, per-batch A tiles
            vg = [None] * ngroups
            A = [None] * bpc
            Et = [None] * bpc
            Eb = [None] * bpc
            OG = [None] * ngroups

            QS = [None] * bpc
            KS = [None] * bpc

            def load_group(g):
                    g0 = g * G
                    if g == 0:
                        # batch 0 gets its own tiles so the first matmuls
                        # depend only on the two small leading DMAs
                        qb0 = qkpool.tile([E, P * L], F32R, tag="qb0")
                        kb0 = qkpool.tile([E, P * L], F32R, tag="kb0")
                        # ACT's own HWDGE queue: dispatches in parallel
                        # with SP's and frees the first matmuls from the
                        # SP queue's in-order completion semaphore
                        nc.scalar.dma_start(kb0[:], k_d[0])
                        nc.scalar.dma_start(qb0[:], q_d[0])
                        qt = qkpool.tile([E, (G - 1) * P * L], F32R, tag="qg")
                        kt = qkpool.tile([E, (G - 1) * P * L], F32R, tag="kg")
                        nc.sync.dma_start(
                            qt[:].rearrange("e (b f) -> e b f", b=G - 1),
                            q_d[1:G].rearrange("b e f -> e b f"),
                        )
                        nc.sync.dma_start(
                            kt[:].rearrange("e (b f) -> e b f", b=G - 1),
                            k_d[1:G].rearrange("b e f -> e b f"),
                        )
                        QS[0], KS[0] = qb0[:], kb0[:]
                        for i in range(1, G):
                            QS[i] = qt[:, (i - 1) * 672 : i * 672]
                            KS[i] = kt[:, (i - 1) * 672 : i * 672]
                    else:
                        qt = qkpool.tile([E, G * P * L], F32R, tag="qg")
                        kt = qkpool.tile([E, G * P * L], F32R, tag="kg")
                        nc.sync.dma_start(
                            qt[:].rearrange("e (b f) -> e b f", b=G),
                            q_d[g0 : g0 + G].rearrange("b e f -> e b f"),
                        )
                        nc.sync.dma_start(
                            kt[:].rearrange("e (b f) -> e b f", b=G),
                            k_d[g0 : g0 + G].rearrange("b e f -> e b f"),
                        )
                        for i in range(G):
                            QS[g0 + i] = qt[:, i * 672 : (i + 1) * 672]
                            KS[g0 + i] = kt[:, i * 672 : (i + 1) * 672]
                    vt = qkpool.tile([L, G * P * EP], BF16, tag="vg")
                    nc.sync.dma_start(
                        vt[:].rearrange("s (b f) -> s b f", b=G),
                        v_d[g0 : g0 + G].rearrange("b s f -> s b f"),
                    )
                    vg[g] = vt

            def stage1_tiles(b):
                """e1 matmuls + exp (ACT or DVE per tile) into A[b].

                Generator: yields after each tile so e2 work of the
                previous batch can interleave — the PE would otherwise
                outrun the exp engines' 745-825ns/tile drain rate and
                stall on the 3-deep PSUM window."""
                qs, ks = QS[b], KS[b]
                at = apool.tile([L, P * P * L], BF16, tag="A")
                A[b] = at
                for j0, nch, eng in TILES0 if b == 0 else TILES:
                    st = spool.tile([L, 2 * SLOT], F32, tag="s")
                    for m in range(nch):
                        jj = j0 + m
                        r, c = divmod(jj, 2)
                        nc.tensor.matmul(
                            st[:, m * SLOT : m * SLOT + CH],
                            lhsT=ks[:, r * L : (r + 1) * L],
                            rhs=qs[:, c * CH : (c + 1) * CH],
                            start=True,
                            stop=True,
                        )
                    src = st[:, 0 : nch * SLOT].rearrange(
                        "s (a c) -> s a c", c=SLOT
                    )[:, :, 0:CH]
                    dst = at[:, j0 * CH : (j0 + nch) * CH].rearrange(
                        "s (a c) -> s a c", c=CH
                    )
                    if eng == "a":
                        nc.scalar.activation(
                            dst, src, mybir.ActivationFunctionType.Exp
                        )
                    else:
                        nc.vector.tensor_scalar(
                            dst.bitcast(I16), src, SCH_A, SCH_B,
                            mybir.AluOpType.mult, mybir.AluOpType.add,
                        )
                    yield

            def stage2_chunks(b):
                """e2 accumulation over p in PSUM (generator, 4 chunks)."""
                g, i = divmod(b, G)
                vt = vg[g]
                at = A[b]
                a3 = at[:].rearrange("s (r f) -> s r f", f=P * L)
                # two separate 1-bank accumulators: the r0-3 group's exps
                # (ACT) and the r4-6 group's (DVE) finish independently
                et0 = e2pool.tile([EP, 384], F32, tag="e2a")
                et1 = e2pool.tile([EP, 288], F32, tag="e2b")
                Et[b] = (et0, et1)
                o0 = et0[:].rearrange("e (r l) -> e r l", l=L)
                o1 = et1[:].rearrange("e (r l) -> e r l", l=L)

                def e2_part(out, r0, r1, p0, p1):
                    for p in range(p0, p1):
                        nc.tensor.matmul(
                            out,
                            lhsT=vt[:, i * 77 + p * EP : i * 77 + (p + 1) * EP],
                            rhs=a3[:, r0:r1, p * L : (p + 1) * L],
                            start=(p == 0),
                            stop=(p == P - 1),
                        )

                # o1 first for the last batch so its evacuation+transposes
                # overlap the o0 matmuls in the drain
                groups = [(o1, 4, 7), (o0, 0, 4)] if b == bpc - 1 else [
                    (o0, 0, 4), (o1, 4, 7)]
                for out, r0, r1 in groups:
                    e2_part(out, r0, r1, 0, 4)
                    yield
                    e2_part(out, r0, r1, 4, P)
                    yield

            def stage3(b):
                g, i = divmod(b, G)
                et0, et1 = Et[b]

                # separate tiles so the r0-3 transposes depend only on the
                # et0 evacuation (and vice versa); GPSIMD can't read PSUM,
                # so the copies split across the two exp engines (Copy
                # shares the ACT table set with Exp)
                eb0 = epool.tile([EP, 4 * L], F32, tag="eb0")
                eb1 = epool.tile([EP, 3 * L], F32, tag="eb1")
                last = b == bpc - 1
                if not last:
                    nc.scalar.activation(
                        eb0[:], et0[:], mybir.ActivationFunctionType.Copy
                    )
                    nc.vector.tensor_copy(eb1[:], et1[:])
                else:
                    # last batch ran e2 o1-first; evacuate in that order
                    nc.vector.tensor_copy(eb1[:], et1[:])
                    nc.scalar.activation(
                        eb0[:], et0[:], mybir.ActivationFunctionType.Copy
                    )
                if b >= bpc - 2:
                    # scores pool is idle for the final batches; keep the
                    # e2 accumulator slot free so the last e2 starts sooner
                    tt = spool.tile([L, P * EP], F32, tag="s")
                else:
                    tt = e2pool.tile([L, P * EP], F32, tag="e2a")

                def transpose_half(rs):
                    for r in rs:
                        src_ = (
                            eb0[:, r * L : (r + 1) * L]
                            if r < 4
                            else eb1[:, (r - 4) * L : (r - 3) * L]
                        )
                        nc.tensor.transpose(
                            tt[:, r * EP : (r + 1) * EP], src_, ident[:]
                        )

                if last:
                    transpose_half(range(4, P))
                    transpose_half(range(4))
                else:
                    transpose_half(range(P))
                t3 = tt[:].rearrange("l (r e) -> l r e", e=EP)
                rd = rdpool.tile([L, P], F32, tag="rd")
                r3 = rd[:].rearrange("l (r u) -> l r u", u=1)
                nc.vector.reciprocal(r3, t3[:, :, E : E + 1])
                if OG[g] is None:
                    og_tile = ogpool.tile([L, G * R], F32, tag="og")
                    OG[g] = og_tile
                og = OG[g]
                dst = og[:, i * R : (i + 1) * R].rearrange("l (e r) -> l r e", r=P)
                rdb = r3.copy()
                rdb.ap = rdb.ap[:-1] + [[0, E]]
                nc.vector.tensor_mul(dst, t3[:, :, 0:E], rdb)

            def flush_group(g):
                g0 = g * G
                if g == ngroups - 1:
                    # split the final flush so the tail only waits on the
                    # last batch's slice
                    nc.sync.dma_start(
                        o_d[g0 : g0 + G - 1].rearrange("b l c -> l b c"),
                        OG[g][:, 0 : (G - 1) * R].rearrange(
                            "l (b c) -> l b c", b=G - 1
                        ),
                    )
                    nc.sync.dma_start(
                        o_d[g0 + G - 1], OG[g][:, (G - 1) * R : G * R]
                    )
                else:
                    nc.sync.dma_start(
                        o_d[g0 : g0 + G].rearrange("b l c -> l b c"),
                        OG[g][:].rearrange("l (b c) -> l b c", b=G),
                    )
                OG[g] = None

            # software pipeline: stage1(b) tiles interleaved with e2
            # chunks of batch b-1 so the exp engines never starve the PE
            def drain(it):
                if it is not None:
                    for _ in it:
                        pass

            for rep in range(repeat):
                load_group(0)
                for b in range(bpc + 1):
                    s1 = s2 = None
                    if b < bpc:
                        g, i = divmod(b, G)
                        if i == 0 and g + 1 < ngroups:
                            load_group(g + 1)
                        s1 = stage1_tiles(b)
                    if b >= 1:
                        s2 = stage2_chunks(b - 1)
                    drain(s1)
                    drain(s2)
                    if b >= 1:
                        stage3(b - 1)
                        if (b - 1) % G == G - 1:
                            flush_group((b - 1) // G)

    nc.compile()
    return nc


def _get_nc(bpc=BPC, repeat=1):
    key = (bpc, repeat)
    if key not in _CACHE:
        _CACHE[key] = _build(bpc, repeat)
    return _CACHE[key]


def _prep(queries, keys, values):
    q = np.asarray(queries, dtype=np.float32)
    k = np.asarray(keys, dtype=np.float32)
    v = np.asarray(values, dtype=np.float32)
    b = q.shape[0]
    # Q2[b, e, p*96+l] = q[b, l, e*7+p]
    q2 = np.ascontiguousarray(
        q.reshape(b, L, E, P).transpose(0, 2, 3, 1).reshape(b, E, P * L)
    )
    # KT[b, e, r*96+s] = k[b, s, e*7+r]
    kt = np.ascontiguousarray(
        k.reshape(b, L, E, P).transpose(0, 2, 3, 1).reshape(b, E, P * L)
    )
    # VT[b, s, p*11+e'] = v[b, s, e'*7+p] for e'<10, 1.0 at e'=10
    v4 = v.reshape(b, L, E, P).transpose(0, 1, 3, 2)  # [b, s, p, e]
    vt = np.concatenate([v4, np.ones((b, L, P, 1), np.float32)], axis=-1)
    vt = np.ascontiguousarray(
        vt.reshape(b, L, P * EP).astype(ml_dtypes.bfloat16)
    )
    return q2, kt, vt


def kernel(queries, keys, values, attn_mask=None, _trace=False):
    nc = _get_nc()
    q2, kt, vt = _prep(queries, keys, values)
    in_maps = []
    for c in range(NCORES):
        s = slice(c * BPC, (c + 1) * BPC)
        in_maps.append({"q2": q2[s], "kt": kt[s], "vt": vt[s]})
    res = None
    for attempt in range(3):
        try:
            res = run_bass_kernel_spmd(
                nc, in_maps, core_ids=list(range(NCORES)), trace=_trace
            )
            break
        except Exception:
            # shared terminal occasionally reports transient NRT device
            # errors; back off and retry
            if attempt == 2:
                raise
            import time as _time

            _time.sleep(15)
    out = np.concatenate([res.results[c]["out"] for c in range(NCORES)], axis=0)
    if _trace:
        kernel.last_exec_time_ns = res.exec_time_ns
        kernel.last_results = res
    return out.astype(np.float32)


# revision 18
# speedup vs baseline: 1.0475x; 1.0022x over previous
"""Grouped-channel attention (CAT FullAttention) Trainium2 kernel.

Math (per batch element b; L=S=96, R=70, E=10, P=7):
  scores[l,s,p,r] = sum_e q[l,e,p] * k[s,e,r]
  A = softmax over (s,p) of scores           (per l, r)
  out[l,e,r]      = sum_{s,p} v[s,e,p] * A[l,s,p,r]

Strategy: pure data parallel over the batch dim (B=256 -> 32 per core x 8
cores). Per batch element on-device:
  e1   (PE) : per r, scores[s,(p,l)] = K_r^T @ Q2, fp32r matmuls, N=336
              chunks written to 512-element-aligned PSUM slots.
  exp       : split across two engines so neither is the bottleneck —
              ACT exp for slots 0-7 (the r0-3 group), DVE for slots 8-13
              (r4-6) via the Schraudolph bit trick: fp32 A bits =
              int32(x * 2^23/ln2 + (127*2^23 - C)).  C centers the
              multiplicative sawtooth at 1 (+-3%); the constant factor
              cancels exactly between softmax numerator and denominator.
  e2   (PE) : per p, E[e',(r,l)] += V_p^T @ A_p accumulated over p in PSUM.
              V carries a ones-channel at e'=10, so E[10,:] is the softmax
              denominator.
  tail      : PSUM evacuation on GPSIMD (Pool), transpose E to [l,(r,e')]
              on PE, reciprocal+broadcast multiply on DVE, contiguous DMA
              of [96,70] per batch.
"""

import sys

if "/opt/trn_rl_repo" not in sys.path:
    sys.path.insert(0, "/opt/trn_rl_repo")

import ml_dtypes
import numpy as np

import concourse.bass as bass
import concourse.bacc as bacc
import concourse.tile as tile
from concourse import mybir, masks
from concourse.bass_utils import run_bass_kernel_spmd

B, L, R = 256, 96, 70
E, P = 10, 7
EP = E + 1  # v channels + ones channel
NCORES = 8
BPC = B // NCORES  # batches per core
G = 4  # batches per DMA group
F32R = mybir.dt.float32r
F32 = mybir.dt.float32
BF16 = mybir.dt.bfloat16
I16 = mybir.dt.int16

# Schraudolph exp in bf16 bits: bits = int16(x * SCH_A + SCH_B) read as
# bf16 gives exp(x) * g, g in [1/1.0303, 1.0303] (centered sawtooth,
# period ln2). The constant factor cancels between softmax num and den.
SCH_A = 184.66428386431385  # 2^7 / ln 2
SCH_B = 16256.0 - 5.5112  # 127 * 2^7 - 2^7*log2(sqrt(1.061451))

_CACHE = {}


def _build(bpc, repeat=1):
    nc = bacc.Bacc("TRN2", target_bir_lowering=False, debug=False, num_devices=NCORES)
    q_d = nc.dram_tensor("q2", [bpc, E, P * L], F32R, kind="ExternalInput").ap()
    k_d = nc.dram_tensor("kt", [bpc, E, P * L], F32R, kind="ExternalInput").ap()
    v_d = nc.dram_tensor("vt", [bpc, L, P * EP], BF16, kind="ExternalInput").ap()
    o_d = nc.dram_tensor("out", [bpc, L, R], F32, kind="ExternalOutput").ap()

    ngroups = bpc // G
    CH = 336  # e1 chunk width: (p,l)=672 split in two, each >=256 for fp32r
    SLOT = 512  # psum chunk slot (one bank)
    # chunk-tile packing: one r per 2-slot tile; slots 0-7 exp'd on ACT
    # (feeds e2 group r0-3), slots 8-13 on DVE via Schraudolph (r4-6).
    # 2-slot tiles with bufs=3 keep 6 PSUM banks but give the PE 3-deep
    # slack over the exp engines, hiding the exp+semaphore latency.
    TILES = [(0, 2, "a"), (2, 2, "a"), (4, 2, "a"), (6, 2, "a"),
             (8, 2, "d"), (10, 2, "d"), (12, 2, "d")]
    # batch 0 leads with a 1-slot region so the first exp fires one cold
    # matmul after the DMA instead of two
    TILES0 = [(0, 1, "a"), (1, 2, "a"), (3, 2, "a"), (5, 2, "a"), (7, 1, "a"),
              (8, 2, "d"), (10, 2, "d"), (12, 2, "d")]

    with tile.TileContext(nc) as tc:
        with (
            tc.tile_pool(name="const", bufs=1) as cpool,
            tc.tile_pool(name="qk", bufs=2) as qkpool,
            tc.tile_pool(name="apool", bufs=3) as apool,
            tc.tile_pool(name="esb", bufs=2) as epool,
            tc.tile_pool(name="rd", bufs=2) as rdpool,
            tc.tile_pool(name="og", bufs=2) as ogpool,
            tc.tile_pool(name="ps_s", bufs=3, space="PSUM") as spool,
            tc.tile_pool(name="ps_e", bufs=1, space="PSUM") as e2pool,
        ):
            ident = cpool.tile([EP, EP], F32)
            masks.make_identity(nc, ident[:])
            # touch Exp once so the ACT table set loads before the exps;
            # it shares ACT with the two leading DMAs issued below
            warm = cpool.tile([1, 1], F32)
            nc.scalar.activation(
                warm[:], ident[0:1, 0:1], mybir.ActivationFunctionType.Exp
            )

            # per-group input tiles, per-batch A tiles
            vg = [None] * ngroups
            A = [None] * bpc
            Et = [None] * bpc
            Eb = [None] * bpc
            OG = [None] * ngroups

            QS = [None] * bpc
            KS = [None] * bpc

            def load_group(g):
                    g0 = g * G
                    if g == 0:
                        # batch 0 gets its own tiles so the first matmuls
                        # depend only on the two small leading DMAs
                        qb0 = qkpool.tile([E, P * L], F32R, tag="qb0")
                        kb0 = qkpool.tile([E, P * L], F32R, tag="kb0")
                        nc.sync.dma_start(kb0[:], k_d[0])
                        nc.sync.dma_start(qb0[:], q_d[0])
                        qt = qkpool.tile([E, (G - 1) * P * L], F32R, tag="qg")
                        kt = qkpool.tile([E, (G - 1) * P * L], F32R, tag="kg")
                        nc.sync.dma_start(
                            qt[:].rearrange("e (b f) -> e b f", b=G - 1),
                            q_d[1:G].rearrange("b e f -> e b f"),
                        )
                        nc.sync.dma_start(
                            kt[:].rearrange("e (b f) -> e b f", b=G - 1),
                            k_d[1:G].rearrange("b e f -> e b f"),
                        )
                        QS[0], KS[0] = qb0[:], kb0[:]
                        for i in range(1, G):
                            QS[i] = qt[:, (i - 1) * 672 : i * 672]
                            KS[i] = kt[:, (i - 1) * 672 : i * 672]
                    else:
                        qt = qkpool.tile([E, G * P * L], F32R, tag="qg")
                        kt = qkpool.tile([E, G * P * L], F32R, tag="kg")
                        nc.sync.dma_start(
                            qt[:].rearrange("e (b f) -> e b f", b=G),
                            q_d[g0 : g0 + G].rearrange("b e f -> e b f"),
                        )
                        nc.sync.dma_start(
                            kt[:].rearrange("e (b f) -> e b f", b=G),
                            k_d[g0 : g0 + G].rearrange("b e f -> e b f"),
                        )
                        for i in range(G):
                            QS[g0 + i] = qt[:, i * 672 : (i + 1) * 672]
                            KS[g0 + i] = kt[:, i * 672 : (i + 1) * 672]
                    vt = qkpool.tile([L, G * P * EP], BF16, tag="vg")
                    nc.sync.dma_start(
                        vt[:].rearrange("s (b f) -> s b f", b=G),
                        v_d[g0 : g0 + G].rearrange("b s f -> s b f"),
                    )
                    vg[g] = vt

            def stage1_tiles(b):
                """e1 matmuls + exp (ACT or DVE per tile) into A[b].

                Generator: yields after each tile so e2 work of the
                previous batch can interleave — the PE would otherwise
                outrun the exp engines' 745-825ns/tile drain rate and
                stall on the 3-deep PSUM window."""
                qs, ks = QS[b], KS[b]
                at = apool.tile([L, P * P * L], BF16, tag="A")
                A[b] = at
                for j0, nch, eng in TILES0 if b == 0 else TILES:
                    st = spool.tile([L, 2 * SLOT], F32, tag="s")
                    for m in range(nch):
                        jj = j0 + m
                        r, c = divmod(jj, 2)
                        nc.tensor.matmul(
                            st[:, m * SLOT : m * SLOT + CH],
                            lhsT=ks[:, r * L : (r + 1) * L],
                            rhs=qs[:, c * CH : (c + 1) * CH],
                            start=True,
                            stop=True,
                        )
                    src = st[:, 0 : nch * SLOT].rearrange(
                        "s (a c) -> s a c", c=SLOT
                    )[:, :, 0:CH]
                    dst = at[:, j0 * CH : (j0 + nch) * CH].rearrange(
                        "s (a c) -> s a c", c=CH
                    )
                    if eng == "a":
                        nc.scalar.activation(
                            dst, src, mybir.ActivationFunctionType.Exp
                        )
                    else:
                        nc.vector.tensor_scalar(
                            dst.bitcast(I16), src, SCH_A, SCH_B,
                            mybir.AluOpType.mult, mybir.AluOpType.add,
                        )
                    yield

            def stage2_chunks(b):
                """e2 accumulation over p in PSUM (generator, 4 chunks)."""
                g, i = divmod(b, G)
                vt = vg[g]
                at = A[b]
                a3 = at[:].rearrange("s (r f) -> s r f", f=P * L)
                # two separate 1-bank accumulators: the r0-3 group's exps
                # (ACT) and the r4-6 group's (DVE) finish independently
                et0 = e2pool.tile([EP, 384], F32, tag="e2a")
                et1 = e2pool.tile([EP, 288], F32, tag="e2b")
                Et[b] = (et0, et1)
                o0 = et0[:].rearrange("e (r l) -> e r l", l=L)
                o1 = et1[:].rearrange("e (r l) -> e r l", l=L)

                def e2_part(out, r0, r1, p0, p1):
                    for p in range(p0, p1):
                        nc.tensor.matmul(
                            out,
                            lhsT=vt[:, i * 77 + p * EP : i * 77 + (p + 1) * EP],
                            rhs=a3[:, r0:r1, p * L : (p + 1) * L],
                            start=(p == 0),
                            stop=(p == P - 1),
                        )

                # o1 first for the last batch so its evacuation+transposes
                # overlap the o0 matmuls in the drain
                groups = [(o1, 4, 7), (o0, 0, 4)] if b == bpc - 1 else [
                    (o0, 0, 4), (o1, 4, 7)]
                for out, r0, r1 in groups:
                    e2_part(out, r0, r1, 0, 4)
                    yield
                    e2_part(out, r0, r1, 4, P)
                    yield

            def stage3(b):
                g, i = divmod(b, G)
                et0, et1 = Et[b]

                # separate tiles so the r0-3 transposes depend only on the
                # et0 evacuation (and vice versa); GPSIMD can't read PSUM,
                # so the copies split across the two exp engines (Copy
                # shares the ACT table set with Exp)
                eb0 = epool.tile([EP, 4 * L], F32, tag="eb0")
                eb1 = epool.tile([EP, 3 * L], F32, tag="eb1")
                last = b == bpc - 1
                if not last:
                    nc.scalar.activation(
                        eb0[:], et0[:], mybir.ActivationFunctionType.Copy
                    )
                    nc.vector.tensor_copy(eb1[:], et1[:])
                else:
                    # last batch ran e2 o1-first; evacuate in that order
                    nc.vector.tensor_copy(eb1[:], et1[:])
                    nc.scalar.activation(
                        eb0[:], et0[:], mybir.ActivationFunctionType.Copy
                    )
                if b >= bpc - 2:
                    # scores pool is idle for the final batches; keep the
                    # e2 accumulator slot free so the last e2 starts sooner
                    tt = spool.tile([L, P * EP], F32, tag="s")
                else:
                    tt = e2pool.tile([L, P * EP], F32, tag="e2a")

                def transpose_half(rs):
                    for r in rs:
                        src_ = (
                            eb0[:, r * L : (r + 1) * L]
                            if r < 4
                            else eb1[:, (r - 4) * L : (r - 3) * L]
                        )
                        nc.tensor.transpose(
                            tt[:, r * EP : (r + 1) * EP], src_, ident[:]
                        )

                if last:
                    transpose_half(range(4, P))
                    transpose_half(range(4))
                else:
                    transpose_half(range(P))
                t3 = tt[:].rearrange("l (r e) -> l r e", e=EP)
                rd = rdpool.tile([L, P], F32, tag="rd")
                r3 = rd[:].rearrange("l (r u) -> l r u", u=1)
                nc.vector.reciprocal(r3, t3[:, :, E : E + 1])
                if OG[g] is None:
                    og_tile = ogpool.tile([L, G * R], F32, tag="og")
                    OG[g] = og_tile
                og = OG[g]
                dst = og[:, i * R : (i + 1) * R].rearrange("l (e r) -> l r e", r=P)
                rdb = r3.copy()
                rdb.ap = rdb.ap[:-1] + [[0, E]]
                nc.vector.tensor_mul(dst, t3[:, :, 0:E], rdb)

            def flush_group(g):
                g0 = g * G
                if g == ngroups - 1:
                    # split the final flush so the tail only waits on the
                    # last batch's slice
                    nc.sync.dma_start(
                        o_d[g0 : g0 + G - 1].rearrange("b l c -> l b c"),
                        OG[g][:, 0 : (G - 1) * R].rearrange(
                            "l (b c) -> l b c", b=G - 1
                        ),
                    )
                    nc.sync.dma_start(
                        o_d[g0 + G - 1], OG[g][:, (G - 1) * R : G * R]
                    )
                else:
                    nc.sync.dma_start(
                        o_d[g0 : g0 + G].rearrange("b l c -> l b c"),
                        OG[g][:].rearrange("l (b c) -> l b c", b=G),
                    )
                OG[g] = None

            # software pipeline: stage1(b) tiles interleaved with e2
            # chunks of batch b-1 so the exp engines never starve the PE
            def drain(it):
                if it is not None:
                    for _ in it:
                        pass

            for rep in range(repeat):
                load_group(0)
                for b in range(bpc + 1):
                    s1 = s2 = None
                    if b < bpc:
                        g, i = divmod(b, G)
                        if i == 0 and g + 1 < ngroups:
                            load_group(g + 1)
                        s1 = stage1_tiles(b)
                    if b >= 1:
                        s2 = stage2_chunks(b - 1)
                    drain(s1)
                    drain(s2)
                    if b >= 1:
                        stage3(b - 1)
                        if (b - 1) % G == G - 1:
                            flush_group((b - 1) // G)

    nc.compile()
    return nc


def _get_nc(bpc=BPC, repeat=1):
    key = (bpc, repeat)
    if key not in _CACHE:
        _CACHE[key] = _build(bpc, repeat)
    return _CACHE[key]


def _prep(queries, keys, values):
    q = np.asarray(queries, dtype=np.float32)
    k = np.asarray(keys, dtype=np.float32)
    v = np.asarray(values, dtype=np.float32)
    b = q.shape[0]
    # Q2[b, e, p*96+l] = q[b, l, e*7+p]
    q2 = np.ascontiguousarray(
        q.reshape(b, L, E, P).transpose(0, 2, 3, 1).reshape(b, E, P * L)
    )
    # KT[b, e, r*96+s] = k[b, s, e*7+r]
    kt = np.ascontiguousarray(
        k.reshape(b, L, E, P).transpose(0, 2, 3, 1).reshape(b, E, P * L)
    )
    # VT[b, s, p*11+e'] = v[b, s, e'*7+p] for e'<10, 1.0 at e'=10
    v4 = v.reshape(b, L, E, P).transpose(0, 1, 3, 2)  # [b, s, p, e]
    vt = np.concatenate([v4, np.ones((b, L, P, 1), np.float32)], axis=-1)
    vt = np.ascontiguousarray(
        vt.reshape(b, L, P * EP).astype(ml_dtypes.bfloat16)
    )
    return q2, kt, vt


def kernel(queries, keys, values, attn_mask=None, _trace=False):
    nc = _get_nc()
    q2, kt, vt = _prep(queries, keys, values)
    in_maps = []
    for c in range(NCORES):
        s = slice(c * BPC, (c + 1) * BPC)
        in_maps.append({"q2": q2[s], "kt": kt[s], "vt": vt[s]})
    res = None
    for attempt in range(3):
        try:
            res = run_bass_kernel_spmd(
                nc, in_maps, core_ids=list(range(NCORES)), trace=_trace
            )
            break
        except Exception:
            # shared terminal occasionally reports transient NRT device
            # errors; back off and retry
            if attempt == 2:
                raise
            import time as _time

            _time.sleep(15)
    out = np.concatenate([res.results[c]["out"] for c in range(NCORES)], axis=0)
    if _trace:
        kernel.last_exec_time_ns = res.exec_time_ns
        kernel.last_results = res
    return out.astype(np.float32)
